# revision 37
# baseline (speedup 1.0000x reference)
"""Trainium2 Bass kernel for DenseFeatureExtractionModule (irregular-pooled VGG).

Sharding: 8 cores = 2 images x 4 row-strips of the 192-grid output (48 rows
each). Each core receives its input strip with enough halo rows to compute
all 10 conv layers locally (no inter-core communication). Out-of-image halo
rows are kept at zero through the layer stack by multiplying edge-band rows
with a per-core row-validity mask, which reproduces SAME-conv zero padding.

All activations + weights are bf16 (fp32 PSUM accumulation; rel err ~1.4e-2
vs the fp32 reference). Graph-conv layers (5-10) use a gather-select path:
per tap, ONE bf16 tensor_copy (4x DVE mode) of the dilation-1 window covering
all cin-tiles stacked along the free axis, plus one copy_predicated overlay
of the dilation-s window (center tap needs no select), then a single set of
N<=512 matmul chains — halving PE work vs dual-dilation. bf16 weights enable
PE fast-weight-load (LDWEIGHTS 107ns vs 195ns fp32). Edge-row masking and
irregular-pool max trees run on GpSimd/DVE off the critical path; DRAM
scratch buffers are never zero-filled: each reader memsets the 8 pad
columns of its own SBUF input tile (GpSimd) and loads only the 192-col
interior, so the DRAM pad bytes are dead. L2/L3 stack the a=-1/a=0 tap rows
into partitions 0-63/64-127 so 6 of 9 taps run as K=128 matmuls; L1/L2
outputs batch 4 rows per DMA. All gather/dense layers draw input (xing) and
gather (gp) tiles from single pools shared across layers — per-layer pool
create/destroy serialized consecutive layers; sharing lets layer n+1's head
(needing only n's first rows) overlap layer n's tail.
Measured: 3.39ms on 8 cores (baseline 5.98-6.22ms).
"""

import numpy as np
import ml_dtypes

import concourse.bacc as bacc
import concourse.bass as bass
import concourse.mybir as mybir
import concourse.tile as tile
from concourse.bass_utils import run_bass_kernel_spmd

F32 = mybir.dt.float32
BF16 = mybir.dt.bfloat16
U8 = mybir.dt.uint8
RELU = mybir.ActivationFunctionType.Relu
MAX = mybir.AluOpType.max
NPBF16 = ml_dtypes.bfloat16

W192 = 192
W384 = 384
PAD = 4  # pad columns for all 192-grid buffers
WP = W192 + 2 * PAD  # 200
A1_WP = W384 + 2  # 386, pad 1

# buffer row counts (per core strips, incl. halo)
CANVAS_ROWS = 180  # batch strip on 384 grid (96 + 2*42)
A1_ROWS = 178
ROWS192 = {"A2": 88, "A3": 86, "A4": 84, "A5": 80, "A6": 76, "A7": 72,
           "A8": 64, "A9": 56, "OUT": 48}
HALO192 = {"A2": 20, "A3": 19, "A4": 18, "A5": 16, "A6": 14, "A7": 12,
           "A8": 8, "A9": 4, "OUT": 0}

_CHANS = [(3, 64), (64, 64), (64, 128), (128, 128), (128, 256),
          (256, 256), (256, 256), (256, 512), (512, 512), (512, 512)]

# (src, dst, Cin, Cout, dils, pool_s, widx) for layers 3..10
LAYERS = [
    ("A2", "A3", 64, 128, (1,), None, 3),
    ("A3", "A4", 128, 128, (1,), 2, 4),
    ("A4", "A5", 128, 256, (1, 2), None, 5),
    ("A5", "A6", 256, 256, (1, 2), None, 6),
    ("A6", "A7", 256, 256, (1, 2), 4, 7),
    ("A7", "A8", 256, 512, (1, 4), None, 8),
    ("A8", "A9", 512, 512, (1, 4), None, 9),
    ("A9", "OUT", 512, 512, (1, 4), None, 10),
]
RB = 8  # output rows per input-tile block (dense 192-grid layers)
GB = 4  # output rows per block in the gather path

TAPS = [(a, b) for a in (-1, 0, 1) for b in (-1, 0, 1)]


def _emit_l1(nc, tc, pools, bufs, params):
    """L1: 1x1 conv over im2col'd input (K=27, M=64), 384 grid."""
    x1, a1 = bufs["X1"], bufs["A1"]
    wsb = pools["const"].tile([27, 64], BF16)
    nc.sync.dma_start(wsb[:], params["w1"][:])
    bsb = pools["const"].tile([64, 1], F32)
    nc.sync.dma_start(bsb[:], params["b1"][:])
    rm = pools["const"].tile([128, CANVAS_ROWS], BF16)
    nc.sync.dma_start(rm[:], params["rm384"][:])
    z1 = pools["const"].tile([64, W384], BF16, name="z1")
    nc.vector.memset(z1[:], 0.0)

    RB1 = 16
    with tc.tile_pool(name="in1", bufs=2) as p_in1, \
         tc.tile_pool(name="out1", bufs=4) as p_out1:
     for i0 in range(0, A1_ROWS, RB1):
        nr = min(RB1, A1_ROWS - i0)
        xt = p_in1.tile([27, RB1, W384], BF16, tag="x1t")
        nc.sync.dma_start(xt[:, :nr, :], x1[:, i0 : i0 + nr, :])
        for j4 in range(0, nr, 4):
            ng = min(4, nr - j4)
            ot = p_out1.tile([64, 4, W384], BF16, tag="o1")
            for j in range(j4, j4 + ng):
                i = i0 + j
                ps = pools["psum"].tile([64, W384], F32, tag="ps")
                nc.tensor.matmul(ps[:], wsb[:], xt[:, j, :], start=True, stop=True)
                if j % 2 == 0:
                    nc.scalar.activation(ot[:, j - j4, :], ps[:], RELU,
                                         bias=bsb[:])
                else:
                    # relu(ps + bias) on the (L1-idle) DVE to unload ACT
                    nc.vector.scalar_tensor_tensor(
                        ot[:, j - j4, :], ps[:], bsb[:], z1[:],
                        mybir.AluOpType.add, MAX)
                if i < 41 or i >= A1_ROWS - 41:
                    nc.vector.tensor_mul(
                        ot[:, j - j4, :], ot[:, j - j4, :],
                        rm[0:64, i + 1 : i + 2].broadcast_to([64, W384]))
            nc.sync.dma_start(a1[:, i0 + j4 : i0 + j4 + ng, 1 : 1 + W384],
                              ot[:, :ng, :])


def _emit_l2(nc, tc, pools, bufs, params):
    """L2 conv (64->64, 384 grid) fused with 2x2 maxpool -> A2 (192 grid).
    The a=-1 and a=0 tap rows are stacked into partitions 0-63 / 64-127 of
    one input tile so 6 of the 9 taps run as K=128 matmuls (6 mm/row not 9)."""
    a1, a2 = bufs["A1"], bufs["A2"]
    wp2 = pools["const"].tile([128, 3, 64], BF16)
    nc.sync.dma_start(wp2[:], params["w2p"][:])
    ws2t = pools["const"].tile([128, 3, 64], BF16)
    nc.sync.dma_start(ws2t[64:128], params["w2s"][:])
    ws2 = ws2t[64:128]
    bsb = pools["const"].tile([64, 1], F32)
    nc.sync.dma_start(bsb[:], params["b2"][:])
    rm = pools["rm192"]

    QB = 8  # A2 rows per block
    with tc.tile_pool(name="in2", bufs=2) as p_in2, \
         tc.tile_pool(name="out2", bufs=4) as p_out2:
     for q0 in range(0, ROWS192["A2"], QB):
        nq = min(QB, ROWS192["A2"] - q0)
        lo = 2 * q0
        xt = p_in2.tile([128, 2 * QB + 2, A1_WP], BF16, tag="x2t")
        nc.gpsimd.memset(xt[:, :, 0:1], 0.0)
        nc.gpsimd.memset(xt[:, :, 1 + W384 :], 0.0)
        # partitions 0-63: a1 row lo+i;  64-127: a1 row lo+i+1
        nc.sync.dma_start(xt[0:64, : 2 * nq + 2, 1 : 1 + W384],
                          a1[:, lo : lo + 2 * nq + 2, 1 : 1 + W384])
        n2 = min(2 * nq + 2, A1_ROWS - lo - 1)
        nc.sync.dma_start(xt[64:128, :n2, 1 : 1 + W384],
                          a1[:, lo + 1 : lo + 1 + n2, 1 : 1 + W384])
        for qg in range(q0, q0 + nq, 4):
            ngq = min(4, q0 + nq - qg)
            mp4 = p_out2.tile([64, 4, W192], BF16, tag="mp2")
            for q in range(qg, qg + ngq):
                o2 = p_out2.tile([64, 2, W384], BF16, tag="o2")
                for r in range(2):
                    R = 2 * (q - q0) + r
                    ps = pools["psum"].tile([64, W384], F32, tag="ps")
                    for bi in range(3):
                        nc.tensor.matmul(ps[:], wp2[:, bi, :],
                                         xt[:, R, bi : bi + W384],
                                         start=(bi == 0), stop=False)
                    for bi in range(3):
                        nc.tensor.matmul(ps[:], ws2[:, bi, :],
                                         xt[64:128, R + 1, bi : bi + W384],
                                         start=False, stop=(bi == 2))
                    nc.scalar.activation(o2[:, r, :], ps[:], RELU, bias=bsb[:])
                o2v = o2[:].rearrange("p r (c t) -> p r c t", t=2)
                cm = p_out2.tile([64, 2, W192], BF16, tag="cm2")
                nc.vector.tensor_tensor(cm[:, 0, :], o2v[:, 0, :, 0], o2v[:, 0, :, 1], MAX)
                nc.vector.tensor_tensor(cm[:, 1, :], o2v[:, 1, :, 0], o2v[:, 1, :, 1], MAX)
                nc.vector.tensor_tensor(mp4[:, q - qg, :], cm[:, 0, :], cm[:, 1, :], MAX)
                if q < 20 or q >= ROWS192["A2"] - 20:
                    nc.vector.tensor_mul(
                        mp4[:, q - qg, :], mp4[:, q - qg, :],
                        rm[0:64, q : q + 1].broadcast_to([64, W192]))
            nc.sync.dma_start(a2[:, qg : qg + ngq, PAD : PAD + W192],
                              mp4[:, :ngq, :])


def _emit_dense192(nc, tc, pools, bufs, params, src, dst, cin, cout, dils,
                   pool_s, widx):
    """Dense 192-grid conv layer (single dilation), optional fused pool."""
    sdram, ddram = bufs[src], bufs[dst]
    rows_out = ROWS192[dst]
    h_out = HALO192[dst]
    nci = (cin + 127) // 128
    nco = (cout + 127) // 128
    off = 20 - h_out
    rm = pools["rm192"]
    msb = pools["m_u8"]

    packed = cin == 64  # stack a=-1/a=0 tap rows into one K=128 tile
    if packed:
        wtp = pools["wres"].tile([128, 3, cout], BF16, name=f"w{widx}p",
                                 tag="wres_0")
        nc.sync.dma_start(wtp[:], params[f"w{widx}p"][:])
        wtst = pools["wres"].tile([128, 3, cout], BF16, name=f"w{widx}s",
                                  tag="wres_1")
        nc.sync.dma_start(wtst[64:128], params[f"w{widx}s"][:])
        wtss = wtst[64:128]
    else:
        wts = []
        for ci in range(nci):
            p = min(128, cin - ci * 128)
            wt = pools["wres"].tile([p, 9, cout], BF16, name=f"w{widx}_{ci}",
                                    tag=f"wres_{ci}")
            nc.sync.dma_start(wt[:], params[f"w{widx}"][ci * 128 : ci * 128 + p])
            wts.append(wt)
    bsb = pools["const"].tile([min(cout, 128), nco], F32, name=f"bsb{widx}")
    nc.sync.dma_start(bsb[:], params[f"b{widx}"][:])

    grp = pool_s if pool_s else 2  # rows per output tile group
    p_xin = pools["xing"]
    if True:
     for j0 in range(0, rows_out, RB):
        rb = min(RB, rows_out - j0)
        xts = []
        if packed:
            xt = p_xin.tile([128, RB + 2, WP], BF16, tag="xin")
            nc.gpsimd.memset(xt[:, :, 0:PAD], 0.0)
            nc.gpsimd.memset(xt[:, :, PAD + W192 :], 0.0)
            nc.sync.dma_start(
                xt[0:64, : rb + 2, PAD : PAD + W192],
                sdram[:, j0 : j0 + rb + 2, PAD : PAD + W192])
            n2 = min(rb + 2, ROWS192[src] - j0 - 1)
            nc.sync.dma_start(
                xt[64:128, :n2, PAD : PAD + W192],
                sdram[:, j0 + 1 : j0 + 1 + n2, PAD : PAD + W192])
            xts.append(xt)
        else:
            for ci in range(nci):
                p = min(128, cin - ci * 128)
                xt = p_xin.tile([p, RB + 2, WP], BF16, tag="xin")
                nc.gpsimd.memset(xt[:, :, 0:PAD], 0.0)
                nc.gpsimd.memset(xt[:, :, PAD + W192 :], 0.0)
                nc.sync.dma_start(
                    xt[:, : rb + 2, PAD : PAD + W192],
                    sdram[ci * 128 : ci * 128 + p, j0 : j0 + rb + 2,
                          PAD : PAD + W192])
                xts.append(xt)
        for co in range(nco):
            pco = min(128, cout - co * 128)
            for g0 in range(0, rb, grp):
                tg = pools["oacc"].tile([pco, grp, W192], BF16, tag="oacc")
                for rp in range(grp // 2):
                    j = j0 + g0 + rp * 2
                    ps = pools["psum"].tile([pco, 2 * W192], F32, tag="ps")
                    if packed:
                        R = g0 + rp * 2
                        for bi in range(3):
                            nc.tensor.matmul(
                                ps[:], wtp[:, bi, co * 128 : co * 128 + pco],
                                xts[0][:, R : R + 2,
                                       PAD - 1 + bi : PAD - 1 + bi + W192],
                                start=(bi == 0), stop=False)
                        for bi in range(3):
                            nc.tensor.matmul(
                                ps[:], wtss[:, bi, co * 128 : co * 128 + pco],
                                xts[0][64:128, R + 1 : R + 3,
                                       PAD - 1 + bi : PAD - 1 + bi + W192],
                                start=False, stop=(bi == 2))
                    else:
                      for ci in range(nci):
                        for ti, (a, b) in enumerate(TAPS):
                            rhs = xts[ci][:, g0 + rp * 2 + 1 + a :
                                          g0 + rp * 2 + 1 + a + 2,
                                          PAD + b : PAD + b + W192]
                            nc.tensor.matmul(
                                ps[:],
                                wts[ci][:, ti, co * 128 : co * 128 + pco],
                                rhs,
                                start=(ci == 0 and ti == 0),
                                stop=(ci == nci - 1 and ti == 8))
                    t1 = tg[:, rp * 2 : rp * 2 + 2, :]
                    psv = ps[:].rearrange("p (r w) -> p r w", w=W192)
                    nc.scalar.activation(t1, psv, RELU, bias=bsb[:pco, co : co + 1])
                    if j < h_out or j + 2 > rows_out - h_out:
                        nc.gpsimd.tensor_mul(
                            t1, t1,
                            rm[:pco, off + j : off + j + 2].unsqueeze(-1)
                            .broadcast_to([pco, 2, W192]))
                j = j0 + g0
                if pool_s == 2:
                    tv = tg[:].rearrange("p r (c t) -> p r c t", t=2)
                    cm = pools["pscr"].tile([pco, 2, W192 // 2], BF16, tag="pcm")
                    nc.vector.tensor_tensor(cm[:], tv[:, :, :, 0], tv[:, :, :, 1], MAX)
                    bm = pools["pscr"].tile([pco, W192 // 2], BF16, tag="pbm")
                    nc.vector.tensor_tensor(bm[:], cm[:, 0, :], cm[:, 1, :], MAX)
                    rep = pools["pscr"].tile([pco, 2, W192], BF16, tag="prep")
                    nc.vector.tensor_copy(
                        rep[:], bm[:].unsqueeze(1).unsqueeze(-1)
                        .broadcast_to([pco, 2, W192 // 2, 2]))
                    nc.vector.copy_predicated(
                        tg[:], msb[:pco, off + j : off + j + 2, :], rep[:])
                nc.sync.dma_start(
                    ddram[co * 128 : co * 128 + pco, j : j + grp,
                          PAD : PAD + W192], tg[:])


def _emit_gather192(nc, tc, pools, bufs, params, src, dst, cin, cout, dils,
                    pool_s, widx):
    """Graph-conv layer via gather-select: one bf16 copy (4x DVE mode) + one
    copy_predicated overlay per tap covering ALL cin-tiles (stacked along the
    free axis), then N<=512 matmul chains. Optional fused s=4 pool."""
    sdram, ddram = bufs[src], bufs[dst]
    rows_out = ROWS192[dst]
    h_out = HALO192[dst]
    s = dils[1]
    dm = s
    nci = (cin + 127) // 128
    nco = (cout + 127) // 128
    off = 20 - h_out
    act_dt = F32 if dst == "OUT" else BF16
    rm = pools["rm192"]
    msb = pools["m_u8"]
    dst_c0 = 0 if dst == "OUT" else PAD
    GBl = 8 if nco <= 2 else 4  # psum banks: nco * ceil(GBl*192/512) <= 8
    S = GBl + 2 * dm  # xin rows per cin-tile slot

    wts = []
    for ci in range(nci):
        wt = pools["wres"].tile([128, 9, cout], BF16, name=f"w{widx}_{ci}",
                                tag=f"wres_{ci}")
        nc.sync.dma_start(wt[:], params[f"w{widx}"][ci * 128 : ci * 128 + 128])
        wts.append(wt)
    bsb = pools["const"].tile([min(cout, 128), nco], F32, name=f"bsb{widx}")
    nc.sync.dma_start(bsb[:], params[f"b{widx}"][:])

    p_xin = pools["xing"]
    p_g = pools["gp"]
    if True:
        for j0 in range(0, rows_out, GBl):
            gb = min(GBl, rows_out - j0)
            ncol = gb * W192
            bounds = [(k * 512, min(ncol, (k + 1) * 512))
                      for k in range((ncol + 511) // 512)]
            xt = p_xin.tile([128, nci * S, WP], BF16, tag="xin")
            nc.gpsimd.memset(xt[:, :, 0:PAD], 0.0)
            nc.gpsimd.memset(xt[:, :, PAD + W192 :], 0.0)
            for ci in range(nci):
                nc.sync.dma_start(
                    xt[:, ci * S : ci * S + gb + 2 * dm, PAD : PAD + W192],
                    sdram[ci * 128 : ci * 128 + 128, j0 : j0 + gb + 2 * dm,
                          PAD : PAD + W192])
            xv = xt[:].rearrange("p (n r) c -> p n r c", n=nci)
            pss = [[pools["psum"].tile([128, hi - lo], F32, tag="ps",
                                       name=f"ps{widx}_{j0}_{co}_{k}")
                    for k, (lo, hi) in enumerate(bounds)] for co in range(nco)]
            for ti, (a, b) in enumerate(TAPS):
                g = p_g.tile([128, nci, GBl, W192], BF16, tag="g3")
                nc.vector.tensor_copy(
                    g[:, :, :gb, :], xv[:, :, dm + a : dm + a + gb,
                                        PAD + b : PAD + b + W192])
                if not (a == 0 and b == 0):
                    nc.vector.copy_predicated(
                        g[:, :, :gb, :],
                        msb[:, off + j0 : off + j0 + gb, :].unsqueeze(1)
                           .broadcast_to([128, nci, gb, W192]),
                        xv[:, :, dm + a * s : dm + a * s + gb,
                           PAD + b * s : PAD + b * s + W192])
                gf = g[:].rearrange("p n r w -> p n (r w)")
                for ci in range(nci):
                    for co in range(nco):
                        pco = min(128, cout - co * 128)
                        for k, (lo, hi) in enumerate(bounds):
                            nc.tensor.matmul(
                                pss[co][k][:pco, :],
                                wts[ci][:, ti, co * 128 : co * 128 + pco],
                                gf[:, ci, lo:hi],
                                start=(ti == 0 and ci == 0),
                                stop=(ti == 8 and ci == nci - 1))
            for co in range(nco):
                pco = min(128, cout - co * 128)
                tg = pools["oacc"].tile([pco, GBl, W192], act_dt, tag="oacc")
                tgf = tg[:].rearrange("p r w -> p (r w)")
                for k, (lo, hi) in enumerate(bounds):
                    nc.scalar.activation(tgf[:, lo:hi], pss[co][k][:pco, :],
                                         RELU, bias=bsb[:pco, co : co + 1])
                if j0 < h_out or j0 + gb > rows_out - h_out:
                    nc.gpsimd.tensor_mul(
                        tg[:, :gb, :], tg[:, :gb, :],
                        rm[:pco, off + j0 : off + j0 + gb].unsqueeze(-1)
                        .broadcast_to([pco, gb, W192]))
                if pool_s == 4:
                    for q0 in range(0, gb, 4):
                        tq = tg[:, q0 : q0 + 4, :]
                        tv = tq.rearrange("p r (c t) -> p r c t", t=4)
                        c1 = pools["pscr"].tile([pco, 4, W192 // 4], BF16, tag="pc1")
                        c2 = pools["pscr"].tile([pco, 4, W192 // 4], BF16, tag="pc2")
                        nc.vector.tensor_tensor(c1[:], tv[:, :, :, 0], tv[:, :, :, 1], MAX)
                        nc.vector.tensor_tensor(c2[:], tv[:, :, :, 2], tv[:, :, :, 3], MAX)
                        nc.vector.tensor_tensor(c1[:], c1[:], c2[:], MAX)
                        r1 = pools["pscr"].tile([pco, W192 // 4], BF16, tag="pr1")
                        r2 = pools["pscr"].tile([pco, W192 // 4], BF16, tag="pr2")
                        nc.vector.tensor_tensor(r1[:], c1[:, 0, :], c1[:, 1, :], MAX)
                        nc.vector.tensor_tensor(r2[:], c1[:, 2, :], c1[:, 3, :], MAX)
                        nc.vector.tensor_tensor(r1[:], r1[:], r2[:], MAX)
                        rep = pools["pscr"].tile([pco, 4, W192], BF16, tag="prep4")
                        nc.vector.tensor_copy(
                            rep[:], r1[:].unsqueeze(1).unsqueeze(-1)
                            .broadcast_to([pco, 4, W192 // 4, 4]))
                        nc.vector.copy_predicated(
                            tq, msb[:pco, off + j0 + q0 : off + j0 + q0 + 4, :],
                            rep[:])
                nc.sync.dma_start(
                    ddram[co * 128 : co * 128 + pco, j0 : j0 + gb,
                          dst_c0 : dst_c0 + W192], tg[:, :gb, :])


def build_program():
    nc = bacc.Bacc()
    params = {}
    params["x1col"] = nc.declare_dram_parameter(
        "x1col", [27, A1_ROWS, W384], BF16, isOutput=False)
    params["w1"] = nc.declare_dram_parameter("w1", [27, 64], BF16, isOutput=False)
    for i, (ci, co) in enumerate(_CHANS):
        if i + 1 in (2, 3):
            params[f"w{i + 1}p"] = nc.declare_dram_parameter(
                f"w{i + 1}p", [128, 3, co], BF16, isOutput=False)
            params[f"w{i + 1}s"] = nc.declare_dram_parameter(
                f"w{i + 1}s", [64, 3, co], BF16, isOutput=False)
        elif i > 0:
            params[f"w{i + 1}"] = nc.declare_dram_parameter(
                f"w{i + 1}", [ci, 9, co], BF16, isOutput=False)
        params[f"b{i + 1}"] = nc.declare_dram_parameter(
            f"b{i + 1}", [min(co, 128), (co + 127) // 128], F32, isOutput=False)
    params["m_u8"] = nc.declare_dram_parameter(
        "m_u8", [128, ROWS192["A2"], W192], U8, isOutput=False)
    params["rm384"] = nc.declare_dram_parameter(
        "rm384", [128, CANVAS_ROWS], BF16, isOutput=False)
    params["rm192"] = nc.declare_dram_parameter(
        "rm192", [128, ROWS192["A2"]], BF16, isOutput=False)

    bufs = {"X1": params["x1col"]}
    bufs["A1"] = nc.dram_tensor("A1", [64, A1_ROWS, A1_WP], BF16)
    for name, cc in (("A2", 64), ("A3", 128), ("A4", 128), ("A5", 256),
                     ("A6", 256), ("A7", 256), ("A8", 512), ("A9", 512)):
        bufs[name] = nc.dram_tensor(name, [cc, ROWS192[name], WP], BF16)
    bufs["OUT"] = nc.declare_dram_parameter(
        "out", [512, ROWS192["OUT"], W192], F32, isOutput=True)

    with tile.TileContext(nc) as tc:
        from contextlib import ExitStack
        with ExitStack() as ctx:
            pools = {}
            for name, kw in (
                ("const", dict(bufs=1)),
                ("oacc", dict(bufs=6)),
                ("pscr", dict(bufs=2)),
                ("psum", dict(bufs=8, space="PSUM")),
            ):
                pools[name] = ctx.enter_context(tc.tile_pool(name=name, **kw))
            # rm192 is needed by L2's edge masking; the big pooling mask only
            # from L4 on — load it after L1/L2 so their input DMAs go first
            pools["rm192"] = pools["const"].tile([128, ROWS192["A2"]], BF16,
                                                 name="rm192_t", tag="rm192")
            nc.sync.dma_start(pools["rm192"][:], params["rm192"][:])
            _emit_l1(nc, tc, pools, bufs, params)
            pools["m_u8"] = pools["const"].tile([128, ROWS192["A2"], W192], U8,
                                                name="m_u8_t", tag="m_u8")
            nc.sync.dma_start(pools["m_u8"][:], params["m_u8"][:])
            _emit_l2(nc, tc, pools, bufs, params)
            with tc.tile_pool(name="wres", bufs=2) as p_wres, \
                 tc.tile_pool(name="xing", bufs=3) as p_xing, \
                 tc.tile_pool(name="gp", bufs=5) as p_gp:
                pools["wres"] = p_wres
                pools["xing"] = p_xing
                pools["gp"] = p_gp
                for lay in LAYERS:
                    if len(lay[4]) == 1:
                        _emit_dense192(nc, tc, pools, bufs, params, *lay)
                    else:
                        _emit_gather192(nc, tc, pools, bufs, params, *lay)
    nc.compile()
    return nc


# ---------------------------------------------------------------- host side

def _upsample_mask(m48):
    return np.repeat(np.repeat(m48, 4, axis=0), 4, axis=1)


def make_core_inputs(inputs, core):
    b, s = core // 4, core % 4
    r0, R0 = 48 * s, 96 * s
    x = np.asarray(inputs["batch"][b], np.float32)  # [3, 384, 384]

    canvas = np.zeros((3, CANVAS_ROWS, W384 + 2), np.float32)
    lo, hi = R0 - 42, R0 + 138
    clo, chi = max(lo, 0), min(hi, W384)
    canvas[:, clo - lo : chi - lo, 1 : 1 + W384] = x[:, clo:chi, :]

    x1col = np.empty((27, A1_ROWS, W384), np.float32)
    for t, (a, bb) in enumerate(TAPS):
        x1col[3 * t : 3 * t + 3] = canvas[:, 1 + a : 1 + a + A1_ROWS,
                                          1 + bb : 1 + bb + W384]

    m192 = _upsample_mask(np.asarray(inputs["pooling_mask"][b, 0]))  # [192,192]
    mbuf = np.zeros((ROWS192["A2"], W192), np.uint8)
    mlo, mhi = r0 - 20, r0 + 68
    cmlo, cmhi = max(mlo, 0), min(mhi, W192)
    mbuf[cmlo - mlo : cmhi - mlo] = m192[cmlo:cmhi].astype(np.uint8)

    rm384 = ((np.arange(CANVAS_ROWS) + R0 - 42 >= 0)
             & (np.arange(CANVAS_ROWS) + R0 - 42 < W384)).astype(np.float32)
    rm192 = ((np.arange(ROWS192["A2"]) + r0 - 20 >= 0)
             & (np.arange(ROWS192["A2"]) + r0 - 20 < W192)).astype(np.float32)

    im = {
        "x1col": x1col.astype(NPBF16),
        "m_u8": np.broadcast_to(mbuf, (128,) + mbuf.shape).copy(),
        "rm384": np.broadcast_to(rm384, (128, CANVAS_ROWS)).astype(NPBF16),
        "rm192": np.broadcast_to(rm192, (128, ROWS192["A2"])).astype(NPBF16),
    }
    w1 = np.asarray(inputs["w1"], np.float32)  # [64, 3, 3, 3]
    w1r = np.empty((27, 64), np.float32)
    for t, (a, bb) in enumerate(TAPS):
        w1r[3 * t : 3 * t + 3] = w1[:, :, a + 1, bb + 1].T
    im["w1"] = w1r.astype(NPBF16)
    for i in range(2, 11):
        w = np.asarray(inputs[f"w{i}"], np.float32)  # [O, I, 3, 3]
        wr = np.ascontiguousarray(
            w.transpose(1, 2, 3, 0).reshape(w.shape[1], 9, w.shape[0]))
        if i in (2, 3):
            co = wr.shape[2]
            wp = np.empty((128, 3, co), np.float32)
            wp[0:64] = wr[:, 0:3]    # taps (a=-1, b)
            wp[64:128] = wr[:, 3:6]  # taps (a=0, b)
            im[f"w{i}p"] = wp.astype(NPBF16)
            im[f"w{i}s"] = np.ascontiguousarray(wr[:, 6:9]).astype(NPBF16)
        else:
            im[f"w{i}"] = wr.astype(NPBF16)
    for i in range(1, 11):
        bv = np.asarray(inputs[f"b{i}"], np.float32)
        im[f"b{i}"] = np.ascontiguousarray(bv.reshape(-1, min(bv.size, 128)).T)
    return im


_NC_CACHE = []


def _get_program():
    if not _NC_CACHE:
        _NC_CACHE.append(build_program())
    return _NC_CACHE[0]


def kernel(**inputs):
    nc = _get_program()
    in_maps = [make_core_inputs(inputs, c) for c in range(8)]
    res = run_bass_kernel_spmd(nc, in_maps, list(range(8)))
    out = np.empty((2, 512, W192, W192), np.float32)
    for c in range(8):
        b, s = c // 4, c % 4
        out[b, :, 48 * s : 48 * s + 48, :] = res.results[c]["out"]
    return out
